# revision 1
# baseline (speedup 1.0000x reference)
"""GNN attention (GAT-style single-target-node) kernel for 8 Trainium2 cores.

Problem:  x [32, 50000, 64], a [128, 1], node_index scalar, adj_mask [50000]
  tgt_score = x[:, idx] @ a[:64]                             # [B]
  e = leaky_relu(tgt_score[:, None] + x @ a[64:], 0.01)      # [B, N]
  attention = softmax(where(adj>0, e, -9e15), axis=1) * adj  # [B, N]

Sharding: data-parallel over batch (32 = 8 cores x 4 batches/core). Each
core computes complete softmax rows, so no cross-core reductions.

Per-core layout: nodes tile as [128 partitions, TK nodes, 64 feats];
scores accumulate into a [128, 400] grid (TF full tiles of TK plus a
[53, 16] tail).  Dot products: elementwise multiply against a replicated
a_src then a grouped free-axis reduce.  The multiply is split between
GPSIMD (k < GK) and DVE (k >= GK) to balance engine load, since DVE also
owns the reduces.  Softmax cross-partition max/sum go through a PE
transpose + ones-matmul broadcast (PE is otherwise idle).
"""

import numpy as np
from contextlib import ExitStack

import jax
from jax.sharding import Mesh, PartitionSpec
from jax.experimental.shard_map import shard_map

import concourse.bass as bass
import concourse.bacc as bacc
import concourse.tile as tile
from concourse.tile import add_dep_helper
from concourse import mybir
from concourse.bass2jax import _bass_exec_p, install_neuronx_cc_hook

B, N, D = 32, 50000, 64
NCORES = 8
BPC = B // NCORES            # batches per core
TK = 64                      # nodes per partition per full tile (divides 384)
TF = 384 // TK               # full tiles, TF * 128 * TK = 49152 nodes
NFULL = TF * 128 * TK        # 49152
KT = 16                      # tail: nodes per partition
PT = (N - NFULL) // KT       # 53 partitions in tail tile
CF = TF * TK                 # 384 full-score columns
COLS = CF + KT               # 400 score columns
GK = 48                      # k < GK multiplies on GPSIMD, rest on DVE
GKT = 12                     # same split for the tail tile
GA = 0                       # k-cols of tree level-1 on GPSIMD (0: the cross-
                             # engine dep inside the tree costs more than it
                             # saves -- GPSIMD's 4-deep FIFO delays level 2)
REDUCE_MODE = "tree"         # "grouped": one reduce_sum; "tree": log2(D) adds
XB = 4                       # x-tile pool depth
PB = 3                       # product-tile pool depth
NEG = -9.0e15

F32 = mybir.dt.float32
AX = mybir.AxisListType
OP = mybir.AluOpType
ACT = mybir.ActivationFunctionType

TRACE = False                # set True (e.g. from test.py) to neuron-profile
LAST_RUN = None              # BassKernelResults of the most recent run

_CACHE = {}


def _build(reps=1):
    nc = bacc.Bacc(trn_type="TRN2", enable_partition_id=False,
                   num_devices=NCORES)
    xs = nc.dram_tensor("xs", [BPC, N, D], F32, kind="ExternalInput").ap()
    tgt_d = nc.dram_tensor("tgtvec", [128, BPC], F32, kind="ExternalInput").ap()
    arep_d = nc.dram_tensor("arep", [128, TK * D], F32, kind="ExternalInput").ap()
    mb_d = nc.dram_tensor("mbgrid", [128, COLS], F32, kind="ExternalInput").ap()
    id_d = nc.dram_tensor("ident", [128, 128], F32, kind="ExternalInput").ap()
    on_d = nc.dram_tensor("onesr", [1, 128], F32, kind="ExternalInput").ap()
    attn = nc.dram_tensor("attn", [BPC, N], F32, kind="ExternalOutput").ap()

    with tile.TileContext(nc) as tc, ExitStack() as ctx:
        singles = ctx.enter_context(tc.tile_pool(name="singles", bufs=1))
        xpool = ctx.enter_context(tc.tile_pool(name="xpool", bufs=XB))
        ppool = ctx.enter_context(tc.tile_pool(name="ppool", bufs=PB))
        spool = ctx.enter_context(tc.tile_pool(name="spool", bufs=2))
        epool = ctx.enter_context(tc.tile_pool(name="epool", bufs=2))
        stat = ctx.enter_context(tc.tile_pool(name="stat", bufs=8))
        pspool = ctx.enter_context(tc.tile_pool(name="ps", bufs=4, space="PSUM"))

        arep_sb = singles.tile([128, TK * D], F32)
        # the pipeline-fill quarters only read the first TK/4 k-columns of the
        # replicated a_src, so land those first and the bulk later
        qcols = (TK // 4) * D
        with tc.high_priority():
            nc.sync.dma_start(out=arep_sb[:, :qcols], in_=arep_d[:, :qcols])
        nc.sync.dma_start(out=arep_sb[:, qcols:], in_=arep_d[:, qcols:])
        arep3 = arep_sb[:].rearrange("p (k d) -> p k d", d=D)
        mb_sb = singles.tile([128, COLS], F32)
        nc.sync.dma_start(out=mb_sb, in_=mb_d)
        tgt_sb = singles.tile([128, BPC], F32)
        nc.sync.dma_start(out=tgt_sb, in_=tgt_d)
        ident = singles.tile([128, 128], F32)
        nc.sync.dma_start(out=ident, in_=id_d)
        onesr = singles.tile([1, 128], F32)
        nc.sync.dma_start(out=onesr, in_=on_d)

        def cross_partition(vec, op):
            """[128,1] per-partition stats -> [1,1] global (PE transpose)."""
            tp = pspool.tile([1, 128], F32, tag="ps")
            nc.tensor.transpose(tp, vec, ident)
            ct = stat.tile([1, 128], F32, tag="ct")
            nc.vector.tensor_copy(ct, tp)
            g1 = stat.tile([1, 1], F32, tag="g1")
            nc.vector.tensor_reduce(g1, ct, axis=AX.X, op=op)
            return g1

        def bcast_partitions(s1, tag):
            """[1,1] scalar -> [128,1] replicated (ones-matmul)."""
            bp = pspool.tile([128, 1], F32, tag="ps")
            nc.tensor.matmul(bp, onesr, s1, start=True, stop=True)
            out = stat.tile([128, 1], F32, tag=tag)
            nc.vector.tensor_copy(out, bp)
            return out

        def mul_split(pr, xt, ar, gk, k):
            if gk > 0:
                nc.gpsimd.tensor_mul(pr[:, :gk, :], xt[:, :gk, :], ar[:, :gk, :])
            if gk < k:
                nc.vector.tensor_mul(pr[:, gk:, :], xt[:, gk:, :], ar[:, gk:, :])

        def dot_reduce(sbcols, pr, ga=0):
            """Sum pr [128, k, 64] over the last axis into sbcols [128, k].
            ga > 0 routes the first ga k-columns of the top tree level to
            GPSIMD to shave the DVE's busy time."""
            if REDUCE_MODE == "grouped":
                nc.vector.reduce_sum(sbcols, pr, axis=AX.X)
                return
            w = D // 2
            if ga > 0:
                nc.gpsimd.tensor_add(pr[:, :ga, 0:w], pr[:, :ga, 0:w],
                                     pr[:, :ga, w:2 * w])
                nc.vector.tensor_add(pr[:, ga:, 0:w], pr[:, ga:, 0:w],
                                     pr[:, ga:, w:2 * w])
                w //= 2
            while w > 1:
                nc.vector.tensor_add(pr[:, :, 0:w], pr[:, :, 0:w],
                                     pr[:, :, w:2 * w])
                w //= 2
            nc.vector.tensor_add(sbcols, pr[:, :, 0], pr[:, :, 1])

        first_tile = True
        quarter_dmas = []
        full_dmas_ordered = 0
        for _ in range(reps):
            for b in range(BPC):
                sb = spool.tile([128, COLS], F32)
                # tail-tile slots with no node behind them: keep them finite so
                # the masked add (-9e15) sends them to zero probability.  (The
                # tail reduce overwrites partitions < PT afterwards.)
                nc.vector.memset(sb[:, CF:], 0.0)
                for t in range(TF):
                    xt = xpool.tile([128, TK, D], F32)
                    src = xs[b, t * 128 * TK:(t + 1) * 128 * TK, :] \
                        .rearrange("(p k) d -> p k d", p=128)
                    if first_tile:
                        # split the very first tile into quarters (own DMAs,
                        # all-DVE multiply) so compute starts ~4x earlier
                        # instead of stalling through one 2 MB DMA + GPSIMD;
                        # high_priority keeps the quarters ahead of the next
                        # tiles' full-size DMAs in the DMA queue
                        first_tile = False
                        q = TK // 4
                        with tc.high_priority():
                            for i in range(4):
                                qd = nc.sync.dma_start(
                                    out=xt[:, i * q:(i + 1) * q, :],
                                    in_=src[:, i * q:(i + 1) * q, :])
                                quarter_dmas.append(qd.ins)
                                pr = ppool.tile([128, q, D], F32, tag="prq")
                                nc.vector.tensor_mul(
                                    pr, xt[:, i * q:(i + 1) * q, :],
                                    arep3[:, :q, :])
                                dot_reduce(
                                    sb[:, t * TK + i * q:t * TK + (i + 1) * q],
                                    pr)
                        continue
                    fd = nc.sync.dma_start(out=xt, in_=src)
                    if quarter_dmas and full_dmas_ordered < 3:
                        # keep the fill quarters ahead of the first full-size
                        # DMAs in the queue (scheduler otherwise reorders)
                        add_dep_helper(fd.ins, quarter_dmas[-1], sync=False,
                                       reason="pipeline fill: quarters first")
                        full_dmas_ordered += 1
                    pr = ppool.tile([128, TK, D], F32)
                    mul_split(pr, xt, arep3, GK, TK)
                    dot_reduce(sb[:, t * TK:(t + 1) * TK], pr, ga=GA)
                # tail tile: 848 nodes = [53 partitions, 16 nodes, 64 feats]
                xt_t = xpool.tile([128, KT, D], F32)
                nc.sync.dma_start(
                    out=xt_t[:PT],
                    in_=xs[b, NFULL:N, :].rearrange("(p k) d -> p k d", p=PT),
                )
                pr_t = ppool.tile([128, KT, D], F32)
                mul_split(pr_t[:PT], xt_t[:PT], arep3[:PT, :KT, :], GKT, KT)
                dot_reduce(sb[:PT, CF:], pr_t[:PT])

                # z = leaky_relu(scores + tgt, 0.01) + mask_bias.  Scores are
                # O(10), so exp(z) cannot overflow fp32 and the usual
                # max-subtraction is unnecessary (softmax is shift-invariant);
                # skipping it removes a whole PE/DVE round-trip per batch.
                z = epool.tile([128, COLS], F32)
                nc.vector.tensor_scalar_add(z, sb, tgt_sb[:, b:b + 1])
                nc.vector.scalar_tensor_tensor(z, z, 0.01, z,
                                               op0=OP.mult, op1=OP.max)
                nc.vector.tensor_add(z, z, mb_sb)

                pb = epool.tile([128, COLS], F32)
                srow = stat.tile([128, 1], F32)
                nc.scalar.activation(pb, z, ACT.Exp, bias=0.0, scale=1.0,
                                     accum_out=srow)
                gsum1 = cross_partition(srow, OP.add)
                rec1 = stat.tile([1, 1], F32)
                nc.vector.reciprocal(rec1, gsum1)
                rec = bcast_partitions(rec1, "rec")
                nc.vector.tensor_scalar_mul(pb, pb, rec)

                nc.sync.dma_start(
                    out=attn[b, 0:NFULL].rearrange("(t p k) -> p t k",
                                                   p=128, k=TK),
                    in_=pb[:, 0:CF].rearrange("p (t k) -> p t k", t=TF),
                )
                nc.sync.dma_start(
                    out=attn[b, NFULL:N].rearrange("(p k) -> p k", k=KT),
                    in_=pb[:PT, CF:],
                )
    nc.compile()
    return nc


def _host_prep(x, a, node_index, adj_mask):
    x = np.asarray(x, dtype=np.float32)
    a = np.asarray(a, dtype=np.float32).reshape(2 * D)
    adj = np.asarray(adj_mask).astype(np.int64)
    idx = int(node_index)
    a_tgt, a_src = a[:D], a[D:]

    tgt = (x[:, idx, :] @ a_tgt).astype(np.float32)          # [B]
    arep = np.tile(a_src, (128, TK)).astype(np.float32)      # [128, TK*D]

    mb = np.full((128, COLS), NEG, np.float32)
    m_full = adj[:NFULL].reshape(TF, 128, TK)
    mb[:, :CF] = np.where(
        m_full.transpose(1, 0, 2).reshape(128, CF) > 0, 0.0, NEG)
    m_tail = adj[NFULL:].reshape(PT, KT)
    mb[:PT, CF:] = np.where(m_tail > 0, 0.0, NEG)
    ident = np.eye(128, dtype=np.float32)
    onesr = np.ones((1, 128), dtype=np.float32)
    return x, tgt, arep, mb, ident, onesr


def _in_maps(x, tgt, arep, mb, ident, onesr):
    maps = []
    for c in range(NCORES):
        tv = np.tile(tgt[c * BPC:(c + 1) * BPC][None, :],
                     (128, 1)).astype(np.float32)
        maps.append({
            "xs": np.ascontiguousarray(x[c * BPC:(c + 1) * BPC]),
            "tgtvec": tv,
            "arep": arep,
            "mbgrid": mb,
            "ident": ident,
            "onesr": onesr,
        })
    return maps


def _runner():
    """Build the Bass program once and wrap its NEFF custom call in a jitted
    shard_map over the 8 cores.  Cached so repeat kernel() calls only pay
    input upload + execution, not re-tracing/compiling."""
    if "runner" in _CACHE:
        return _CACHE["runner"]
    install_neuronx_cc_hook()
    nc = _CACHE.setdefault("nc", _build())
    in_names, out_names, out_avals, zero_shapes = [], [], [], []
    for alloc in nc.m.functions[0].allocations:
        if not isinstance(alloc, mybir.MemoryLocationSet):
            continue
        name = alloc.memorylocations[0].name
        if alloc.kind == "ExternalInput":
            in_names.append(name)
        elif alloc.kind == "ExternalOutput":
            out_names.append(name)
            shape = tuple(alloc.tensor_shape)
            dtype = mybir.dt.np(alloc.dtype)
            out_avals.append(jax.core.ShapedArray(shape, dtype))
            zero_shapes.append((shape, dtype))

    def _body(*args):
        return tuple(_bass_exec_p.bind(
            *args,
            out_avals=tuple(out_avals),
            in_names=tuple(in_names + out_names),
            out_names=tuple(out_names),
            lowering_input_output_aliases=(),
            sim_require_finite=True,
            sim_require_nnan=True,
            nc=nc,
        ))

    mesh = Mesh(np.asarray(jax.devices()[:NCORES]), ("core",))
    nin = len(in_names) + len(out_names)
    sharded = jax.jit(shard_map(
        _body, mesh=mesh,
        in_specs=(PartitionSpec("core"),) * nin,
        out_specs=(PartitionSpec("core"),) * len(out_names),
        check_rep=False))
    _CACHE["runner"] = (sharded, in_names, out_names, zero_shapes)
    return _CACHE["runner"]


def kernel(x, a, node_index, adj_mask):
    global LAST_RUN
    prep = _host_prep(x, a, node_index, adj_mask)
    maps = _in_maps(*prep)
    sharded, in_names, out_names, zero_shapes = _runner()
    # concat of the 8 per-core xs shards is exactly the full x — skip the copy
    ins = [prep[0] if nm == "xs" else
           np.concatenate([m[nm] for m in maps], axis=0) for nm in in_names]
    zeros = [np.zeros((NCORES * s[0], *s[1:]), d) for s, d in zero_shapes]
    outs = sharded(*ins, *zeros)
    LAST_RUN = outs
    attn = np.asarray(outs[out_names.index("attn")])  # [NCORES*BPC, N]
    return attn.reshape(B, N)



# revision 7
# speedup vs baseline: 2.8292x; 2.8292x over previous
"""GNN attention (GAT-style single-target-node) kernel for 8 Trainium2 cores.

Problem:  x [32, 50000, 64], a [128, 1], node_index scalar, adj_mask [50000]
  tgt_score = x[:, idx] @ a[:64]                             # [B]
  e = leaky_relu(tgt_score[:, None] + x @ a[64:], 0.01)      # [B, N]
  attention = softmax(where(adj>0, e, -9e15), axis=1) * adj  # [B, N]

Sharding: data-parallel over batch (32 = 8 cores x 4 batches/core). Each
core computes complete softmax rows, so no cross-core reductions.

Per-core layout: nodes tile as [128 partitions, TK nodes, 64 feats];
scores accumulate into a [128, 400] grid (TF full tiles of TK plus a
[53, 16] tail).  Dot products: elementwise multiply against a replicated
a_src then a grouped free-axis reduce.  The multiply is split between
GPSIMD (k < GK) and DVE (k >= GK) to balance engine load, since DVE also
owns the reduces.  Softmax cross-partition max/sum go through a PE
transpose + ones-matmul broadcast (PE is otherwise idle).
"""

import numpy as np
from contextlib import ExitStack

import jax
from jax.sharding import Mesh, PartitionSpec
from jax.experimental.shard_map import shard_map

import concourse.bass as bass
import concourse.bacc as bacc
import concourse.tile as tile
from concourse.tile import add_dep_helper
from concourse import mybir
from concourse.bass2jax import _bass_exec_p, install_neuronx_cc_hook

B, N, D = 32, 50000, 64
NCORES = 8
BPC = B // NCORES            # batches per core
TK = 128                     # nodes per partition per full tile (divides 384)
TF = 384 // TK               # full tiles, TF * 128 * TK = 49152 nodes
NFULL = TF * 128 * TK        # 49152
KT = 16                      # tail: nodes per partition
PT = (N - NFULL) // KT       # 53 partitions in tail tile
CF = TF * TK                 # 384 full-score columns
COLS = CF + KT               # 400 score columns
GK = 52                      # k < GK multiplies on GPSIMD, rest on DVE
GKT = 6                      # same split for the tail tile
GA = 0                       # k-cols of tree level-1 on GPSIMD (0: the cross-
                             # engine dep inside the tree costs more than it
                             # saves -- GPSIMD's 4-deep FIFO delays level 2)
REDUCE_MODE = "tree"         # "grouped": one reduce_sum; "tree": log2(D) adds
XB = 4                       # x-tile pool depth
PB = 3                       # product-tile pool depth
NEG = -9.0e15

F32 = mybir.dt.float32
F16 = mybir.dt.float16       # x / a_src travel and multiply in fp16: halves
                             # HBM traffic and doubles DVE tensor_tensor
                             # throughput (2x_1P mode); scores + softmax stay
                             # f32, so the only error is fp16 input rounding
AX = mybir.AxisListType
OP = mybir.AluOpType
ACT = mybir.ActivationFunctionType

TRACE = False                # set True (e.g. from test.py) to neuron-profile
LAST_RUN = None              # BassKernelResults of the most recent run

_CACHE = {}


def _build(reps=1):
    nc = bacc.Bacc(trn_type="TRN2", enable_partition_id=False,
                   num_devices=NCORES)
    xs = nc.dram_tensor("xs", [BPC, N, D], F16, kind="ExternalInput").ap()
    tgt_d = nc.dram_tensor("tgtvec", [128, BPC], F32, kind="ExternalInput").ap()
    arep_d = nc.dram_tensor("arep", [128, TK * D], F16, kind="ExternalInput").ap()
    mb_d = nc.dram_tensor("mbgrid", [128, COLS], F32, kind="ExternalInput").ap()
    id_d = nc.dram_tensor("ident", [128, 128], F32, kind="ExternalInput").ap()
    on_d = nc.dram_tensor("onesr", [1, 128], F32, kind="ExternalInput").ap()
    attn = nc.dram_tensor("attn", [BPC, N], F32, kind="ExternalOutput").ap()

    with tile.TileContext(nc) as tc, ExitStack() as ctx:
        singles = ctx.enter_context(tc.tile_pool(name="singles", bufs=1))
        xpool = ctx.enter_context(tc.tile_pool(name="xpool", bufs=XB))
        ppool = ctx.enter_context(tc.tile_pool(name="ppool", bufs=PB))
        spool = ctx.enter_context(tc.tile_pool(name="spool", bufs=2))
        epool = ctx.enter_context(tc.tile_pool(name="epool", bufs=2))
        stat = ctx.enter_context(tc.tile_pool(name="stat", bufs=8))
        pspool = ctx.enter_context(tc.tile_pool(name="ps", bufs=4, space="PSUM"))

        arep_sb = singles.tile([128, TK * D], F16)
        # the pipeline-fill quarters only read the first TK/4 k-columns of the
        # replicated a_src, so land those first and the bulk later
        qcols = (TK // 4) * D
        with tc.high_priority():
            nc.sync.dma_start(out=arep_sb[:, :qcols], in_=arep_d[:, :qcols])
        nc.sync.dma_start(out=arep_sb[:, qcols:], in_=arep_d[:, qcols:])
        arep3 = arep_sb[:].rearrange("p (k d) -> p k d", d=D)
        mb_sb = singles.tile([128, COLS], F32)
        nc.sync.dma_start(out=mb_sb, in_=mb_d)
        tgt_sb = singles.tile([128, BPC], F32)
        nc.sync.dma_start(out=tgt_sb, in_=tgt_d)
        ident = singles.tile([128, 128], F32)
        nc.sync.dma_start(out=ident, in_=id_d)
        onesr = singles.tile([1, 128], F32)
        nc.sync.dma_start(out=onesr, in_=on_d)

        def cross_partition(vec, op):
            """[128,1] per-partition stats -> [1,1] global (PE transpose)."""
            tp = pspool.tile([1, 128], F32, tag="ps")
            nc.tensor.transpose(tp, vec, ident)
            ct = stat.tile([1, 128], F32, tag="ct")
            nc.vector.tensor_copy(ct, tp)
            g1 = stat.tile([1, 1], F32, tag="g1")
            nc.vector.tensor_reduce(g1, ct, axis=AX.X, op=op)
            return g1

        def bcast_partitions(s1, tag):
            """[1,1] scalar -> [128,1] replicated (ones-matmul)."""
            bp = pspool.tile([128, 1], F32, tag="ps")
            nc.tensor.matmul(bp, onesr, s1, start=True, stop=True)
            out = stat.tile([128, 1], F32, tag=tag)
            nc.vector.tensor_copy(out, bp)
            return out

        def mul_split(pr, xt, ar, gk, k):
            if gk > 0:
                nc.gpsimd.tensor_mul(pr[:, :gk, :], xt[:, :gk, :], ar[:, :gk, :])
            if gk < k:
                nc.vector.tensor_mul(pr[:, gk:, :], xt[:, gk:, :], ar[:, gk:, :])

        def dot_reduce(sbcols, pr, ga=0):
            """Sum pr [128, k, 64] over the last axis into sbcols [128, k].
            ga > 0 routes the first ga k-columns of the top tree level to
            GPSIMD to shave the DVE's busy time."""
            if REDUCE_MODE == "grouped":
                nc.vector.reduce_sum(sbcols, pr, axis=AX.X)
                return
            w = D // 2
            if ga > 0:
                nc.gpsimd.tensor_add(pr[:, :ga, 0:w], pr[:, :ga, 0:w],
                                     pr[:, :ga, w:2 * w])
                nc.vector.tensor_add(pr[:, ga:, 0:w], pr[:, ga:, 0:w],
                                     pr[:, ga:, w:2 * w])
                w //= 2
            while w > 1:
                nc.vector.tensor_add(pr[:, :, 0:w], pr[:, :, 0:w],
                                     pr[:, :, w:2 * w])
                w //= 2
            nc.vector.tensor_add(sbcols, pr[:, :, 0], pr[:, :, 1])

        first_tile = True
        quarter_dmas = []
        full_dmas_ordered = 0
        for _ in range(reps):
            for b in range(BPC):
                sb = spool.tile([128, COLS], F32)
                # tail-tile slots with no node behind them: keep them finite so
                # the masked add (-9e15) sends them to zero probability.  (The
                # tail reduce overwrites partitions < PT afterwards.)
                nc.vector.memset(sb[:, CF:], 0.0)
                for t in range(TF):
                    xt = xpool.tile([128, TK, D], F16)
                    src = xs[b, t * 128 * TK:(t + 1) * 128 * TK, :] \
                        .rearrange("(p k) d -> p k d", p=128)
                    if first_tile:
                        # split the very first tile into quarters (own DMAs,
                        # all-DVE multiply) so compute starts ~4x earlier
                        # instead of stalling through one 2 MB DMA + GPSIMD;
                        # high_priority keeps the quarters ahead of the next
                        # tiles' full-size DMAs in the DMA queue
                        first_tile = False
                        q = TK // 4
                        with tc.high_priority():
                            for i in range(4):
                                qd = nc.sync.dma_start(
                                    out=xt[:, i * q:(i + 1) * q, :],
                                    in_=src[:, i * q:(i + 1) * q, :])
                                quarter_dmas.append(qd.ins)
                                pr = ppool.tile([128, q, D], F16, tag="prq")
                                nc.vector.tensor_mul(
                                    pr, xt[:, i * q:(i + 1) * q, :],
                                    arep3[:, :q, :])
                                dot_reduce(
                                    sb[:, t * TK + i * q:t * TK + (i + 1) * q],
                                    pr)
                        continue
                    fd = nc.sync.dma_start(out=xt, in_=src)
                    if quarter_dmas and full_dmas_ordered < 3:
                        # keep the fill quarters ahead of the first full-size
                        # DMAs in the queue (scheduler otherwise reorders)
                        add_dep_helper(fd.ins, quarter_dmas[-1], sync=False,
                                       reason="pipeline fill: quarters first")
                        full_dmas_ordered += 1
                    pr = ppool.tile([128, TK, D], F16)
                    mul_split(pr, xt, arep3, GK, TK)
                    dot_reduce(sb[:, t * TK:(t + 1) * TK], pr, ga=GA)
                # tail tile: 848 nodes = [53 partitions, 16 nodes, 64 feats]
                xt_t = xpool.tile([128, KT, D], F16)
                nc.sync.dma_start(
                    out=xt_t[:PT],
                    in_=xs[b, NFULL:N, :].rearrange("(p k) d -> p k d", p=PT),
                )
                pr_t = ppool.tile([128, KT, D], F16)
                mul_split(pr_t[:PT], xt_t[:PT], arep3[:PT, :KT, :], GKT, KT)
                dot_reduce(sb[:PT, CF:], pr_t[:PT])

                # z = leaky_relu(scores + tgt, 0.01) + mask_bias.  Scores are
                # O(10), so exp(z) cannot overflow fp32 and the usual
                # max-subtraction is unnecessary (softmax is shift-invariant);
                # skipping it removes a whole PE/DVE round-trip per batch.
                z = epool.tile([128, COLS], F32)
                nc.vector.tensor_scalar_add(z, sb, tgt_sb[:, b:b + 1])
                nc.vector.scalar_tensor_tensor(z, z, 0.01, z,
                                               op0=OP.mult, op1=OP.max)
                nc.vector.tensor_add(z, z, mb_sb)

                pb = epool.tile([128, COLS], F32)
                srow = stat.tile([128, 1], F32)
                nc.scalar.activation(pb, z, ACT.Exp, bias=0.0, scale=1.0,
                                     accum_out=srow)
                gsum1 = cross_partition(srow, OP.add)
                rec1 = stat.tile([1, 1], F32)
                nc.vector.reciprocal(rec1, gsum1)
                rec = bcast_partitions(rec1, "rec")
                nc.vector.tensor_scalar_mul(pb, pb, rec)

                nc.sync.dma_start(
                    out=attn[b, 0:NFULL].rearrange("(t p k) -> p t k",
                                                   p=128, k=TK),
                    in_=pb[:, 0:CF].rearrange("p (t k) -> p t k", t=TF),
                )
                nc.sync.dma_start(
                    out=attn[b, NFULL:N].rearrange("(p k) -> p k", k=KT),
                    in_=pb[:PT, CF:],
                )
    nc.compile()
    return nc


def _host_prep(x, a, node_index, adj_mask):
    x = np.asarray(x, dtype=np.float32)
    a = np.asarray(a, dtype=np.float32).reshape(2 * D)
    adj = np.asarray(adj_mask).astype(np.int64)
    idx = int(node_index)
    a_tgt, a_src = a[:D], a[D:]

    tgt = (x[:, idx, :] @ a_tgt).astype(np.float32)          # [B]
    x = x.astype(np.float16)                                 # device-side dtype
    arep = np.tile(a_src, (128, TK)).astype(np.float16)      # [128, TK*D]

    mb = np.full((128, COLS), NEG, np.float32)
    m_full = adj[:NFULL].reshape(TF, 128, TK)
    mb[:, :CF] = np.where(
        m_full.transpose(1, 0, 2).reshape(128, CF) > 0, 0.0, NEG)
    m_tail = adj[NFULL:].reshape(PT, KT)
    mb[:PT, CF:] = np.where(m_tail > 0, 0.0, NEG)
    ident = np.eye(128, dtype=np.float32)
    onesr = np.ones((1, 128), dtype=np.float32)
    return x, tgt, arep, mb, ident, onesr


def _in_maps(x, tgt, arep, mb, ident, onesr):
    maps = []
    for c in range(NCORES):
        tv = np.tile(tgt[c * BPC:(c + 1) * BPC][None, :],
                     (128, 1)).astype(np.float32)
        maps.append({
            "xs": np.ascontiguousarray(x[c * BPC:(c + 1) * BPC]),
            "tgtvec": tv,
            "arep": arep,
            "mbgrid": mb,
            "ident": ident,
            "onesr": onesr,
        })
    return maps


def _runner():
    """Build the Bass program once and wrap its NEFF custom call in a jitted
    shard_map over the 8 cores.  Cached so repeat kernel() calls only pay
    input upload + execution, not re-tracing/compiling."""
    if "runner" in _CACHE:
        return _CACHE["runner"]
    install_neuronx_cc_hook()
    nc = _CACHE.setdefault("nc", _build())
    in_names, out_names, out_avals, zero_shapes = [], [], [], []
    for alloc in nc.m.functions[0].allocations:
        if not isinstance(alloc, mybir.MemoryLocationSet):
            continue
        name = alloc.memorylocations[0].name
        if alloc.kind == "ExternalInput":
            in_names.append(name)
        elif alloc.kind == "ExternalOutput":
            out_names.append(name)
            shape = tuple(alloc.tensor_shape)
            dtype = mybir.dt.np(alloc.dtype)
            out_avals.append(jax.core.ShapedArray(shape, dtype))
            zero_shapes.append((shape, dtype))

    def _body(*args):
        return tuple(_bass_exec_p.bind(
            *args,
            out_avals=tuple(out_avals),
            in_names=tuple(in_names + out_names),
            out_names=tuple(out_names),
            lowering_input_output_aliases=(),
            sim_require_finite=True,
            sim_require_nnan=True,
            nc=nc,
        ))

    mesh = Mesh(np.asarray(jax.devices()[:NCORES]), ("core",))
    nin = len(in_names) + len(out_names)
    sharded = jax.jit(shard_map(
        _body, mesh=mesh,
        in_specs=(PartitionSpec("core"),) * nin,
        out_specs=(PartitionSpec("core"),) * len(out_names),
        check_rep=False))
    _CACHE["runner"] = (sharded, in_names, out_names, zero_shapes)
    return _CACHE["runner"]


def kernel(x, a, node_index, adj_mask):
    global LAST_RUN
    prep = _host_prep(x, a, node_index, adj_mask)
    maps = _in_maps(*prep)
    sharded, in_names, out_names, zero_shapes = _runner()
    # concat of the 8 per-core xs shards is exactly the full x — skip the copy
    ins = [prep[0] if nm == "xs" else
           np.concatenate([m[nm] for m in maps], axis=0) for nm in in_names]
    zeros = [np.zeros((NCORES * s[0], *s[1:]), d) for s, d in zero_shapes]
    outs = sharded(*ins, *zeros)
    LAST_RUN = outs
    attn = np.asarray(outs[out_names.index("attn")])  # [NCORES*BPC, N]
    return attn.reshape(B, N)



# revision 9
# speedup vs baseline: 3.1708x; 1.1207x over previous
"""GNN attention (GAT-style single-target-node) kernel for 8 Trainium2 cores.

Problem:  x [32, 50000, 64], a [128, 1], node_index scalar, adj_mask [50000]
  tgt_score = x[:, idx] @ a[:64]                             # [B]
  e = leaky_relu(tgt_score[:, None] + x @ a[64:], 0.01)      # [B, N]
  attention = softmax(where(adj>0, e, -9e15), axis=1) * adj  # [B, N]

Sharding: data-parallel over batch (32 = 8 cores x 4 batches/core), the 4
batches paired into 2 batch-pairs.  Each core computes complete softmax
rows, so no cross-core reductions.

All dot products run on the otherwise-idle PE: the host lays x out as
xh[pair, bi*64+d, col] (fp16, both batches of a pair stacked on the
contraction axis), each 128-node chunk of xh is the *stationary* operand
[K=128, M=128 nodes], and a tiny constant a-matrix [128, 2] streams as the
moving operand, so LDWEIGHTS itself is the data pass (128 values/cycle) and
out = [128 nodes, 2 batches] lands dense in PSUM.  Host column permutation
col = c*128 + p <-> node p*391 + c makes the final attention write
contiguous per partition.  DVE only does the short softmax tail, reading
scores straight out of PSUM; fp16 x halves HBM traffic (the roofline).
"""

import numpy as np
from contextlib import ExitStack

import jax
from jax.sharding import Mesh, PartitionSpec
from jax.experimental.shard_map import shard_map

import concourse.bass as bass
import concourse.bass_isa as bass_isa
import concourse.bacc as bacc
import concourse.tile as tile
from concourse.tile import add_dep_helper
from concourse import mybir
from concourse.bass2jax import _bass_exec_p, install_neuronx_cc_hook

B, N, D = 32, 50000, 64
NCORES = 8
BPC = B // NCORES            # batches per core
PAIRS = BPC // 2             # batch-pairs per core
CHUNKS = 391                 # 128-node chunks per batch: 128*391 = 50048
PADN = 128 * CHUNKS          # padded node count (48 pad nodes)
W = 2 * CHUNKS               # score-grid cols per pair: (chunk, batch) pairs
BANK = 512                   # f32 cols per PSUM bank
CPB = BANK // 2              # chunks per PSUM bank (256)
TILE_F = 16384               # xh cols per DMA tile (4 MB fp16, 128 chunks)
CPT = TILE_F // 128          # chunks per full tile (128)
NMAIN = 127 * CHUNKS         # nodes covered by the [127, 391] output DMA
NTAIL = N - NMAIN            # 343 nodes in partition 127
XB = 4                       # x-tile pool depth
RING_SPLIT = False           # alternate x-tile DMAs across sync/scalar HWDGE
TILES_F = (16384, 16384, 16384, 896)   # per-pair DMA tile widths (cols)
NEG = -9.0e15

F32 = mybir.dt.float32
F16 = mybir.dt.float16
AX = mybir.AxisListType
OP = mybir.AluOpType
ACT = mybir.ActivationFunctionType

TRACE = False
LAST_RUN = None

_CACHE = {}


def _build(reps=1, hw_loop=1, mode="full"):
    """reps: python-unrolled kernel bodies; hw_loop > 1 additionally wraps
    them in a hardware For_i loop (total bodies = reps * hw_loop) so timing
    NEFFs can amortize the ~100 ms (+/- tens of ms) axon dispatch jitter over
    hundreds of bodies without exploding the instruction count.

    mode: diagnostic bodies for attributing HW time -- "full" (the real
    kernel), "dma" (x DMA stream only), "pe" (matmuls+softmax from a static
    SBUF tile, no x DMAs), "both" (DMA stream + static-tile matmuls: no
    data dependency between the two streams)."""
    nc = bacc.Bacc(trn_type="TRN2", enable_partition_id=False,
                   num_devices=NCORES)
    xs = nc.dram_tensor("xs", [PAIRS, 128, PADN], F16,
                        kind="ExternalInput").ap()
    amov_d = nc.dram_tensor("amov", [128, 2], F16, kind="ExternalInput").ap()
    tgtg_d = nc.dram_tensor("tgtg", [PAIRS, 128, W], F32,
                            kind="ExternalInput").ap()
    mb_d = nc.dram_tensor("mbgrid", [128, W], F32, kind="ExternalInput").ap()
    attn = nc.dram_tensor("attn", [BPC, N], F32, kind="ExternalOutput").ap()

    tiles_f = list(TILES_F)
    assert sum(tiles_f) == PADN

    with tile.TileContext(nc) as tc, ExitStack() as ctx:
        singles = ctx.enter_context(tc.tile_pool(name="singles", bufs=1))
        xpool = ctx.enter_context(tc.tile_pool(name="xpool", bufs=XB))
        gpool = ctx.enter_context(tc.tile_pool(name="gpool", bufs=2))
        epool = ctx.enter_context(tc.tile_pool(name="epool", bufs=2))
        stat = ctx.enter_context(tc.tile_pool(name="stat", bufs=8))
        psco = ctx.enter_context(tc.tile_pool(name="psco", bufs=2,
                                              space="PSUM"))

        amov_sb = singles.tile([128, 2], F16)
        with tc.high_priority():
            nc.sync.dma_start(out=amov_sb, in_=amov_d)
        mb_sb = singles.tile([128, W], F32)
        nc.scalar.dma_start(out=mb_sb, in_=mb_d)
        tgtg_sb = singles.tile([128, PAIRS * W], F32)
        for j in range(PAIRS):
            nc.scalar.dma_start(out=tgtg_sb[:, j * W:(j + 1) * W],
                              in_=tgtg_d[j])
        tgtg_v = [tgtg_sb[:, j * W:(j + 1) * W] for j in range(PAIRS)]
        state = {"first_tile": True}

        def body():
            for _ in range(reps):
                _one_rep()

        xstat = None
        if mode in ("pe", "both"):
            xstat = singles.tile([128, TILE_F], F16)
            nc.vector.memset(xstat, 0.0)

        def _one_rep():
            for j in range(PAIRS):
                # --- load xh tiles and run one matmul per 128-node chunk ---
                xts = []
                f0 = 0
                for t, tf in enumerate(tiles_f):
                    f1 = f0 + tf
                    if mode == "pe":
                        f0 = f1
                        continue
                    xt = xpool.tile([128, tf], F16)
                    if state["first_tile"]:
                        # quarter the very first DMA so PE starts ~4x earlier
                        state["first_tile"] = False
                        q = (f1 - f0) // 4
                        with tc.high_priority():
                            for i in range(4):
                                nc.sync.dma_start(
                                    out=xt[:, i * q:(i + 1) * q],
                                    in_=xs[j, :, f0 + i * q:f0 + (i + 1) * q])
                    else:
                        eng = nc.scalar if (RING_SPLIT and t % 2) else nc.sync
                        eng.dma_start(out=xt, in_=xs[j, :, f0:f1])
                    xts.append((f0, xt))
                    f0 = f1
                if mode == "dma":
                    continue

                ps0 = psco.tile([128, BANK], F32, tag="sc0")
                ps1 = psco.tile([128, BANK], F32, tag="sc1")
                ps = [ps0, ps1]
                for c in range(CHUNKS):
                    col = c * 128
                    if mode in ("pe", "both"):
                        cm = col % (TILE_F - 128)
                        lhsT = xstat[:, cm:cm + 128]
                    else:
                        tf0, xt = next((f0, x) for f0, x in reversed(xts)
                                       if f0 <= col)
                        lhsT = xt[:, col - tf0:col - tf0 + 128]
                    bank, cb = divmod(c, CPB)
                    nc.tensor.matmul(ps[bank][:, 2 * cb:2 * cb + 2],
                                     lhsT, amov_sb, start=True, stop=True)

                # --- softmax tail, straight out of PSUM ---
                # z = leaky_relu(scores + tgt, 0.01) + mask_bias.  Scores are
                # O(10): exp cannot overflow fp32, so no max-subtraction.
                z = epool.tile([128, W], F32)
                nc.vector.tensor_add(z[:, :BANK], ps[0], tgtg_v[j][:, :BANK])
                nc.vector.tensor_add(z[:, BANK:], ps[1][:, :W - BANK],
                                     tgtg_v[j][:, BANK:])
                nc.vector.scalar_tensor_tensor(z, z, 0.01, z,
                                               op0=OP.mult, op1=OP.max)
                nc.vector.tensor_add(z, z, mb_sb)

                pbc = epool.tile([128, 2, CHUNKS], F32, tag="pbc")
                for bi in range(2):
                    b = 2 * j + bi
                    # exp with per-partition row sums; global sum + broadcast
                    # in ONE idle-GPSIMD op (daisy-chain all-reduce) instead
                    # of the PE-transpose / ones-matmul round trip.
                    srow = stat.tile([128, 1], F32)
                    nc.scalar.activation(pbc[:, bi, :], z[:, bi::2], ACT.Exp,
                                         bias=0.0, scale=1.0, accum_out=srow)
                    gsum = stat.tile([128, 1], F32, tag="gsum")
                    nc.gpsimd.partition_all_reduce(gsum, srow, 128,
                                                   bass_isa.ReduceOp.add)
                    rec = stat.tile([128, 1], F32, tag="rec")
                    nc.vector.reciprocal(rec, gsum)
                    nc.vector.tensor_scalar_mul(pbc[:, bi, :], pbc[:, bi, :],
                                                rec)
                    nc.scalar.dma_start(
                        out=attn[b, 0:NMAIN].rearrange("(p c) -> p c",
                                                       c=CHUNKS),
                        in_=pbc[0:127, bi, :])
                    nc.scalar.dma_start(
                        out=attn[b, NMAIN:N].rearrange("(o c) -> o c", o=1),
                        in_=pbc[127:128, bi, 0:NTAIL])

        if hw_loop > 1:
            with tc.For_i(0, hw_loop):
                body()
        else:
            body()
    nc.compile()
    return nc


def _host_prep(x, a, node_index, adj_mask):
    x = np.asarray(x, dtype=np.float32)
    a = np.asarray(a, dtype=np.float32).reshape(2 * D)
    adj = np.asarray(adj_mask).astype(np.int64)
    idx = int(node_index)
    a_tgt, a_src = a[:D], a[D:]

    tgt = (x[:, idx, :] @ a_tgt).astype(np.float32)          # [B]

    # xh[pair, bi*64+d, c*128+p] = x[2*pair+bi, p*391+c, d]  (fp16, 0-padded)
    perm = (np.arange(CHUNKS)[:, None] + np.arange(128)[None, :] * CHUNKS)
    perm = perm.ravel()                                      # col -> node id
    xt16 = np.ascontiguousarray(x.transpose(0, 2, 1), dtype=np.float16)
    xtp = np.concatenate(
        [xt16, np.zeros((B, D, PADN - N), np.float16)], axis=2)
    xh = np.ascontiguousarray(xtp[:, :, perm]).reshape(B // 2, 128, PADN)

    amov = np.zeros((128, 2), np.float16)
    amov[0:D, 0] = a_src
    amov[D:2 * D, 1] = a_src

    # mask-bias in (p, 2c+bi) layout; pad nodes (>= N) get NEG
    mb1 = np.full(PADN, NEG, np.float32)
    valid = perm < N
    mb1[valid] = np.where(adj[perm[valid]] > 0, 0.0, NEG)
    mbg = np.repeat(mb1.reshape(CHUNKS, 128).T[:, :, None], 2,
                    axis=2).reshape(128, W)

    # tgt grid [B//2 pairs, 128, W]: value tgt[2*pair+bi] in cols 2c+bi
    tgtg = np.ascontiguousarray(np.broadcast_to(
        tgt.reshape(B // 2, 1, 1, 2), (B // 2, 128, CHUNKS, 2))
    ).reshape(B // 2, 128, W)

    return xh, amov, tgtg, mbg


def _in_maps(xh, amov, tgtg, mbg):
    maps = []
    for c in range(NCORES):
        maps.append({
            "xs": xh[c * PAIRS:(c + 1) * PAIRS],
            "amov": amov,
            "tgtg": tgtg[c * PAIRS:(c + 1) * PAIRS],
            "mbgrid": mbg,
        })
    return maps


def _runner():
    """Build the Bass program once and wrap its NEFF custom call in a jitted
    shard_map over the 8 cores."""
    if "runner" in _CACHE:
        return _CACHE["runner"]
    install_neuronx_cc_hook()
    nc = _CACHE.setdefault("nc", _build())
    in_names, out_names, out_avals, zero_shapes = [], [], [], []
    for alloc in nc.m.functions[0].allocations:
        if not isinstance(alloc, mybir.MemoryLocationSet):
            continue
        name = alloc.memorylocations[0].name
        if alloc.kind == "ExternalInput":
            in_names.append(name)
        elif alloc.kind == "ExternalOutput":
            out_names.append(name)
            shape = tuple(alloc.tensor_shape)
            dtype = mybir.dt.np(alloc.dtype)
            out_avals.append(jax.core.ShapedArray(shape, dtype))
            zero_shapes.append((shape, dtype))

    def _body(*args):
        return tuple(_bass_exec_p.bind(
            *args,
            out_avals=tuple(out_avals),
            in_names=tuple(in_names + out_names),
            out_names=tuple(out_names),
            lowering_input_output_aliases=(),
            sim_require_finite=True,
            sim_require_nnan=True,
            nc=nc,
        ))

    mesh = Mesh(np.asarray(jax.devices()[:NCORES]), ("core",))
    nin = len(in_names) + len(out_names)
    sharded = jax.jit(shard_map(
        _body, mesh=mesh,
        in_specs=(PartitionSpec("core"),) * nin,
        out_specs=(PartitionSpec("core"),) * len(out_names),
        check_rep=False))
    _CACHE["runner"] = (sharded, in_names, out_names, zero_shapes)
    return _CACHE["runner"]


def kernel(x, a, node_index, adj_mask):
    global LAST_RUN
    prep = _host_prep(x, a, node_index, adj_mask)
    maps = _in_maps(*prep)
    sharded, in_names, out_names, zero_shapes = _runner()
    # concat of the 8 per-core xs/tgtg shards is exactly the full arrays
    full = {"xs": prep[0], "tgtg": prep[2]}
    ins = [full[nm] if nm in full else
           np.concatenate([m[nm] for m in maps], axis=0) for nm in in_names]
    zeros = [np.zeros((NCORES * s[0], *s[1:]), d) for s, d in zero_shapes]
    outs = sharded(*ins, *zeros)
    LAST_RUN = outs
    attn = np.asarray(outs[out_names.index("attn")])  # [NCORES*BPC, N]
    return attn.reshape(B, N)
